# revision 34
# baseline (speedup 1.0000x reference)
"""Trainium2 Bass kernel for a dense transformer block (nn_Block_7713761264306).

Sharding: 8 cores = 4 batches x 2 query-halves. Each core computes K/V over the
full sequence for its batch, but runs only its 1024 query rows through
attention and the FFN. The query half is selected by rotating the token axis
host-side (exact: no mask, softmax is permutation-invariant over keys).
No collectives.

Device layout: activations are kept feature-on-partition ([D, tokens]) so every
linear layer is a direct PE matmul (lhsT = weights, rhs = activations^T) with
no on-device transposes. LayerNorm statistics are computed with ones-vector
matmuls on the tensor engine; [1,N] row -> [128,N] broadcasts use K=1 matmuls.
Softmax runs in S^T layout (keys on partitions, queries free); denominators
come from a ones-column appended to V in the PV matmul. All matmuls are bf16
with fp32 PSUM accumulation; LN1's gamma/beta are folded into W_ap host-side.
o and h round-trip through DRAM so SBUF pool lifetimes nest (LIFO).
"""

import numpy as np
import ml_dtypes

import concourse.bass as bass
import concourse.mybir as mybir
import concourse.tile as tile
from concourse.bass import ts
from concourse.bass_utils import run_bass_kernel_spmd

BF16 = mybir.dt.bfloat16
F32 = mybir.dt.float32
bf16 = ml_dtypes.bfloat16

B, T, D, H, HS, FF = 4, 2048, 1024, 16, 64, 4096
P = 128
DC = D // P          # 8 feature chunks
FC = FF // P         # 32 ffn chunks
TKC = T // P         # 16 key/token chunks
Tq = 1024            # queries per core
NT = T // 512        # 4 column tiles over full seq
NQ = Tq // 512       # 2 column tiles over queries
N_CORES = 8
EPS = 1e-5

AF = mybir.ActivationFunctionType
ALU = mybir.AluOpType


def build_nc(cap=True):
    nc = bass.Bass()
    io = {}
    io["xt"] = nc.dram_tensor("xt", [DC, P, T], BF16, kind="ExternalInput")
    io["wq"] = nc.dram_tensor("wq", [DC, P, DC, P], BF16, kind="ExternalInput")
    io["wk"] = nc.dram_tensor("wk", [DC, P, DC, P], BF16, kind="ExternalInput")
    io["wv"] = nc.dram_tensor("wv", [DC, P, D], BF16, kind="ExternalInput")
    io["bqkv"] = nc.dram_tensor("bqkv", [P, 2 * DC], F32, kind="ExternalInput")
    io["bv"] = nc.dram_tensor("bv", [D], F32, kind="ExternalInput")
    io["wproj"] = nc.dram_tensor("wproj", [DC, P, DC, P], BF16,
                                 kind="ExternalInput")
    io["bproj"] = nc.dram_tensor("bproj", [P, DC], F32, kind="ExternalInput")
    io["w1"] = nc.dram_tensor("w1", [FC, P, DC, P], BF16, kind="ExternalInput")
    io["b1"] = nc.dram_tensor("b1", [P, FC], F32, kind="ExternalInput")
    io["w2"] = nc.dram_tensor("w2", [DC, P, FC, P], BF16, kind="ExternalInput")
    io["b2"] = nc.dram_tensor("b2", [P, DC], F32, kind="ExternalInput")
    io["g2"] = nc.dram_tensor("g2", [P, DC], F32, kind="ExternalInput")
    io["bln2"] = nc.dram_tensor("bln2", [P, DC], F32, kind="ExternalInput")
    io["out"] = nc.dram_tensor("out", [DC, P, Tq], F32, kind="ExternalOutput")

    with tile.TileContext(nc) as tc:
        _emit(nc, tc, io)
    nc.finalize()
    if cap:
        _cap_waits(nc)
    return nc


def _cap_waits(nc, keep_types=()):
    """This toolchain's walrus accepts only one sync-wait command per compute
    instruction; hoist extra waits into preceding same-engine NoOps."""
    cnt = 0
    for fn in nc.m.functions:
        for blk in fn.blocks:
            new = []
            for inst in blk.instructions:
                si = getattr(inst, "sync_info", None)
                if si is not None and len(si.on_wait) > 1 \
                        and type(inst).__name__ not in keep_types:
                    waits = list(si.on_wait)
                    for w in waits[:-1]:
                        cnt += 1
                        nop = mybir.InstNoOp(
                            name=f"{inst.name}-w{cnt}", ins=[], outs=[])
                        nop.engine = inst.engine
                        nop.sync_info = mybir.SyncInfo(on_wait=[w],
                                                       on_update=[])
                        new.append(nop)
                    inst.sync_info = mybir.SyncInfo(
                        on_wait=[waits[-1]], on_update=list(si.on_update))
                new.append(inst)
            blk.instructions = new
    return cnt


def _emit(nc, tc, io):
    xT_d = io["xt"]

    consts = tc.alloc_tile_pool(name="consts", bufs=1)

    # ------------------------- constants -------------------------
    bqkv_s = consts.tile([P, 2 * DC], F32)
    nc.sync.dma_start(out=bqkv_s, in_=io["bqkv"][:])
    bproj_s = consts.tile([P, DC], F32)
    nc.sync.dma_start(out=bproj_s, in_=io["bproj"][:])
    b1_s = consts.tile([P, FC], F32)
    nc.sync.dma_start(out=b1_s, in_=io["b1"][:])
    b2_s = consts.tile([P, DC], F32)
    nc.sync.dma_start(out=b2_s, in_=io["b2"][:])
    g2_s = consts.tile([P, DC], F32)
    nc.sync.dma_start(out=g2_s, in_=io["g2"][:])
    bln2_s = consts.tile([P, DC], F32)
    nc.sync.dma_start(out=bln2_s, in_=io["bln2"][:])
    bvB = consts.tile([P, D], F32)
    nc.sync.dma_start(out=bvB, in_=io["bv"][:].partition_broadcast(P))

    invD = consts.tile([P, 1], BF16)
    nc.vector.memset(invD, 1.0 / D)
    onesK1 = consts.tile([1, P], BF16)
    nc.vector.memset(onesK1, 1.0)
    eps_t = consts.tile([1, 1], F32)
    nc.vector.memset(eps_t, EPS)

    def ln_stats_bcast(pp_stat, pp_b, rpool, src_bf, src_sq, ncols, rB, murB,
                       tag):
        """Per-512 col block: mean/E[x^2] over partitions via PE ones-matmul,
        row math, then broadcast 1/sd and mu/sd to [P, 512] via K=1 matmul."""
        for kt in range(ncols // 512):
            ps_mu = pp_stat.tile([1, 512], F32, tag="st",
                                 name=f"psmu{tag}{kt}")
            for c in range(DC):
                nc.tensor.matmul(ps_mu, invD, src_bf[:, c, ts(kt, 512)],
                                 start=(c == 0), stop=(c == DC - 1))
            ps_sq = pp_stat.tile([1, 512], F32, tag="st",
                                 name=f"pssq{tag}{kt}")
            for c in range(DC):
                nc.tensor.matmul(ps_sq, invD, src_sq[:, c, ts(kt, 512)],
                                 start=(c == 0), stop=(c == DC - 1))
            mu = rpool.tile([1, 512], F32, tag="rowf", name=f"mu{tag}{kt}")
            nc.vector.tensor_copy(out=mu, in_=ps_mu)
            var = rpool.tile([1, 512], F32, tag="rowf", name=f"var{tag}{kt}")
            nc.vector.tensor_mul(var, mu, mu)
            nc.vector.tensor_sub(var, ps_sq, var)
            sd = rpool.tile([1, 512], F32, tag="rowf", name=f"sd{tag}{kt}")
            nc.scalar.activation(out=sd, in_=var, func=AF.Sqrt, bias=eps_t,
                                 scale=1.0)
            r = rpool.tile([1, 512], F32, tag="rowf", name=f"r{tag}{kt}")
            nc.vector.reciprocal(out=r, in_=sd)
            rbfr = rpool.tile([1, 512], BF16, tag="rowb", name=f"rbfr{tag}{kt}")
            nc.vector.tensor_copy(out=rbfr, in_=r)
            mur = rpool.tile([1, 512], F32, tag="rowf", name=f"mur{tag}{kt}")
            nc.vector.tensor_mul(mur, mu, r)
            rbfm = rpool.tile([1, 512], BF16, tag="rowb", name=f"rbfm{tag}{kt}")
            nc.vector.tensor_copy(out=rbfm, in_=mur)
            bp1 = pp_b.tile([P, 512], F32, tag="bc", name=f"bp1{tag}{kt}")
            nc.tensor.matmul(bp1, onesK1, rbfr)
            nc.scalar.copy(out=rB[:, ts(kt, 512)], in_=bp1)
            bp2 = pp_b.tile([P, 512], F32, tag="bc", name=f"bp2{tag}{kt}")
            nc.tensor.matmul(bp2, onesK1, rbfm)
            nc.scalar.copy(out=murB[:, ts(kt, 512)], in_=bp2)

    # attention-output chunks, produced in D, consumed by proj in E
    poch = tc.alloc_tile_pool(name="poch", bufs=8)
    # pbig: time-shared 4MB-class slots (tag s4):
    #   A: xT, xsq, xln | D: xln, P(even), P(odd)
    pbig = tc.alloc_tile_pool(name="pbig", bufs=3)

    # ============ Phase A: x^T load, LN1 -> xln ============
    prbm = tc.alloc_tile_pool(name="prbm", bufs=1)
    rowsA = tc.alloc_tile_pool(name="rowsA", bufs=4)
    ppA_st = tc.alloc_tile_pool(name="ppA_st", bufs=2, space="PSUM")
    ppA_b = tc.alloc_tile_pool(name="ppA_b", bufs=2, space="PSUM")

    xT = pbig.tile([P, DC, T], BF16, tag="s4", name="xT")
    for c in range(DC):
        for hh in range(2):
            nc.sync.dma_start(out=xT[:, c, ts(hh, T // 2)],
                              in_=xT_d[c][:, ts(hh, T // 2)])
    xsq = pbig.tile([P, DC, T], BF16, tag="s4", name="xsq")
    for c in range(DC):
        nc.vector.tensor_mul(xsq[:, c, :], xT[:, c, :], xT[:, c, :])
    rB = prbm.tile([P, T], BF16, tag="rb", name="rB")
    murB = prbm.tile([P, T], BF16, tag="mb", name="murB")
    ln_stats_bcast(ppA_st, ppA_b, rowsA, xT, xsq, T, rB, murB, "1")
    xln = pbig.tile([P, DC, T], BF16, tag="s4", name="xln")
    for c in range(DC):
        nc.vector.tensor_mul(xln[:, c, :], xT[:, c, :], rB)
        nc.vector.tensor_sub(xln[:, c, :], xln[:, c, :], murB)

    ppA_b.release()
    ppA_st.release()
    rowsA.release()
    prbm.release()

    # ===== Phases C+D fused: V, then per head-pair K,Q -> scores -> exp
    # -> PV -> normalize, software-pipelined so PE matmuls overlap ACT exp.
    pvaug = tc.alloc_tile_pool(name="pvaug", bufs=1)
    pwkv = tc.alloc_tile_pool(name="pwkv", bufs=3)
    pKp = tc.alloc_tile_pool(name="pKp", bufs=2)
    pQp = tc.alloc_tile_pool(name="pQp", bufs=2)
    poun = tc.alloc_tile_pool(name="poun", bufs=2)
    prb = tc.alloc_tile_pool(name="prb", bufs=2)
    ppD_mm = tc.alloc_tile_pool(name="ppD_mm", bufs=2, space="PSUM")
    ppD_s = tc.alloc_tile_pool(name="ppD_s", bufs=2, space="PSUM")
    ppD_ob = tc.alloc_tile_pool(name="ppD_ob", bufs=2, space="PSUM")
    pwv = tc.alloc_tile_pool(name="pwv", bufs=1)

    def make_kq(hp):
        """Produce the pair's K^T [P, T] and Q^T [P, Tq] chunks."""
        wkj = pwkv.tile([P, DC, P], BF16, tag="w", name=f"wkj{hp}")
        nc.sync.dma_start(out=wkj, in_=io["wk"][hp])
        Kp = pKp.tile([P, T], BF16, tag="kp", name=f"kp{hp}")
        for npair in range(NT // 2):
            ps2 = [ppD_mm.tile([P, 512], F32, tag="mm",
                               name=f"psk{hp}_{npair}_{n}") for n in range(2)]
            for k in range(DC):
                for n in range(2):
                    nc.tensor.matmul(
                        ps2[n], wkj[:, k, :],
                        xln[:, k, ts(2 * npair + n, 512)],
                        start=(k == 0), stop=(k == DC - 1))
            for n in range(2):
                nc.vector.tensor_scalar_add(
                    Kp[:, ts(2 * npair + n, 512)], ps2[n],
                    bqkv_s[:, DC + hp:DC + hp + 1])
        wqj = pwkv.tile([P, DC, P], BF16, tag="w", name=f"wqj{hp}")
        nc.sync.dma_start(out=wqj, in_=io["wq"][hp])
        Qp = pQp.tile([P, Tq], BF16, tag="qp", name=f"qp{hp}")
        ps2 = [ppD_mm.tile([P, 512], F32, tag="mm", name=f"psq{hp}_{n}")
               for n in range(NQ)]
        for k in range(DC):
            for n in range(NQ):
                nc.tensor.matmul(ps2[n], wqj[:, k, :], xln[:, k, ts(n, 512)],
                                 start=(k == 0), stop=(k == DC - 1))
        for n in range(NQ):
            nc.vector.tensor_scalar_add(Qp[:, ts(n, 512)], ps2[n],
                                        bqkv_s[:, hp:hp + 1])
        return Kp, Qp

    och_tiles = []
    kq = make_kq(0)

    wv_t = pwv.tile([P, DC, D], BF16, name="wv_t")
    for c in range(DC):
        nc.sync.dma_start(out=wv_t[:, c, :], in_=io["wv"][c])
    v_aug = pvaug.tile([P, TKC, H * (HS + 1)], BF16, name="v_aug")
    v4 = v_aug.rearrange("p i (h e) -> p i h e", e=HS + 1)
    nc.vector.memset(v4[:, :, :, HS:HS + 1], 1.0)
    for i in range(TKC):
        ps = [ppD_mm.tile([P, 512], F32, tag="mm", name=f"psv{i}_{n}")
              for n in range(NQ)]
        for k in range(DC):
            for n in range(NQ):
                nc.tensor.matmul(ps[n], xln[:, k, ts(i, P)],
                                 wv_t[:, k, ts(n, 512)],
                                 start=(k == 0), stop=(k == DC - 1))
        for n in range(NQ):
            dst = v4[:, i, n * DC:(n + 1) * DC, 0:HS]
            nc.vector.tensor_add(dst,
                                 ps[n].rearrange("p (h d) -> p h d", d=HS),
                                 bvB[:, ts(n, 512)].rearrange(
                                     "p (h d) -> p h d", d=HS))
    pwv.release()

    for hp in range(DC):
        Kp, Qp = kq
        p_tiles = {}
        for local in (0, 1):
            p_tiles[local] = pbig.tile([P, TKC, Tq], BF16, tag="s4",
                                       name=f"pt{hp}_{local}")
        # scores + exp, two heads interleaved (row-tiled on PE)
        for kc in range(TKC):
            ps_s = {}
            for local in (0, 1):
                lo = local * HS
                ps_s[local] = ppD_s.tile([P, Tq], F32, tag="s",
                                         name=f"pss{hp}_{kc}_{local}")
                for n in range(NQ):
                    nc.tensor.matmul(ps_s[local][:, ts(n, 512)],
                                     Kp[lo:lo + HS, ts(kc, P)],
                                     Qp[lo:lo + HS, ts(n, 512)])
            for local in (0, 1):
                nc.scalar.activation(out=p_tiles[local][:, kc, :],
                                     in_=ps_s[local], func=AF.Exp,
                                     scale=float(1.0 / np.sqrt(HS)))
        # produce next pair's K/Q while ACT exps this pair
        if hp + 1 < DC:
            kq = make_kq(hp + 1)
        # PV + denominators
        oun = poun.tile([P, Tq], BF16, tag="ou", name=f"oun{hp}")
        recips_bf = {}
        for local in (0, 1):
            h = 2 * hp + local
            rc = prb.tile([1, Tq], F32, tag="rc", name=f"rc{hp}_{local}")
            for qt in range(NQ):
                po = ppD_ob.tile([HS + 1, 512], F32, tag="ob",
                                 name=f"po{h}_{qt}")
                for kc in range(TKC):
                    nc.tensor.matmul(
                        po, v_aug[:, kc, h * (HS + 1):(h + 1) * (HS + 1)],
                        p_tiles[local][:, kc, ts(qt, 512)],
                        start=(kc == 0), stop=(kc == TKC - 1))
                nc.vector.tensor_copy(
                    out=oun[local * HS:(local + 1) * HS, ts(qt, 512)],
                    in_=po[0:HS, :])
                nc.vector.reciprocal(out=rc[:, ts(qt, 512)],
                                     in_=po[HS:HS + 1, :])
            rcb = prb.tile([1, Tq], BF16, tag="rcb", name=f"rcb{hp}_{local}")
            nc.vector.tensor_copy(out=rcb, in_=rc)
            recips_bf[local] = rcb
        # broadcast each head's reciprocal row to its 64-partition half
        och = poch.tile([P, Tq], BF16, tag="oc", name=f"och{hp}")
        for n in range(NQ):
            rbp = ppD_ob.tile([P, 512], F32, tag="ob", name=f"rbp{hp}_{n}")
            for local in (0, 1):
                nc.tensor.matmul(rbp[local * HS:(local + 1) * HS, :],
                                 onesK1[:, 0:HS],
                                 recips_bf[local][:, ts(n, 512)])
            rbs = prb.tile([P, 512], BF16, tag="rbs", name=f"rbs{hp}_{n}")
            nc.vector.tensor_copy(out=rbs, in_=rbp)
            nc.vector.tensor_mul(och[:, ts(n, 512)], oun[:, ts(n, 512)], rbs)
        och_tiles.append(och)

    ppD_ob.release()
    ppD_s.release()
    ppD_mm.release()
    prb.release()
    poun.release()
    pQp.release()
    pKp.release()
    pwkv.release()
    pvaug.release()
    pbig.release()

    # ============ Phase E: proj + residual -> h (SBUF), n-outer so LN2
    # stats can start after the first column half is complete ============
    ph = tc.alloc_tile_pool(name="ph", bufs=1)
    pxq = tc.alloc_tile_pool(name="pxq", bufs=1)
    pwproj = tc.alloc_tile_pool(name="pwproj", bufs=8)
    ppE = tc.alloc_tile_pool(name="ppE", bufs=8, space="PSUM")

    h_t = ph.tile([P, DC, Tq], F32, name="h_t")
    wpj = []
    for j in range(DC):
        wj = pwproj.tile([P, DC, P], BF16, tag="w", name=f"wpj{j}")
        nc.sync.dma_start(out=wj, in_=io["wproj"][j])
        wpj.append(wj)
    xq_t = pxq.tile([P, DC, Tq], BF16, name="xq_t")
    for c in range(DC):
        nc.sync.dma_start(out=xq_t[:, c, :], in_=xT_d[c][:, 0:Tq])
    for n in range(NQ):
        for j in range(DC):
            psn = ppE.tile([P, 512], F32, tag="mm", name=f"psp{j}_{n}")
            for k in range(DC):
                nc.tensor.matmul(psn, wpj[j][:, k, :],
                                 och_tiles[k][:, ts(n, 512)],
                                 start=(k == 0), stop=(k == DC - 1))
            nc.vector.scalar_tensor_tensor(
                out=h_t[:, j, ts(n, 512)], in0=psn,
                scalar=bproj_s[:, j:j + 1], in1=xq_t[:, j, ts(n, 512)],
                op0=ALU.add, op1=ALU.add)
    ppE.release()
    pwproj.release()
    pxq.release()

    # ============ Phase F: LN2 + gelu -> g ============
    pg = tc.alloc_tile_pool(name="pg", bufs=1)
    phb = tc.alloc_tile_pool(name="phb", bufs=1)
    phsq = tc.alloc_tile_pool(name="phsq", bufs=1)
    pcen = tc.alloc_tile_pool(name="pcen", bufs=1)
    rowsF = tc.alloc_tile_pool(name="rowsF", bufs=4)
    pr2 = tc.alloc_tile_pool(name="pr2", bufs=1)
    ppF_st = tc.alloc_tile_pool(name="ppF_st", bufs=2, space="PSUM")
    ppF_b = tc.alloc_tile_pool(name="ppF_b", bufs=2, space="PSUM")

    g_t = pg.tile([P, DC, Tq], BF16, name="g_t")
    hb = phb.tile([P, DC, Tq], BF16, name="hb")
    hsq = phsq.tile([P, DC, Tq], BF16, name="hsq")
    for kt in range(NQ):
        for c in range(DC):
            nc.vector.tensor_copy(out=hb[:, c, ts(kt, 512)],
                                  in_=h_t[:, c, ts(kt, 512)])
            nc.vector.tensor_mul(hsq[:, c, ts(kt, 512)],
                                 hb[:, c, ts(kt, 512)],
                                 hb[:, c, ts(kt, 512)])
    r2B = pr2.tile([P, Tq], BF16, tag="rb", name="r2B")
    mur2B = pr2.tile([P, Tq], BF16, tag="mb", name="mur2B")
    ln_stats_bcast(ppF_st, ppF_b, rowsF, hb, hsq, Tq, r2B, mur2B, "2")
    cen = pcen.tile([P, DC, Tq], BF16, name="cen")
    for kt in range(NQ):
        for c in range(DC):
            nc.vector.tensor_mul(cen[:, c, ts(kt, 512)],
                                 hb[:, c, ts(kt, 512)], r2B[:, ts(kt, 512)])
            nc.vector.tensor_sub(cen[:, c, ts(kt, 512)],
                                 cen[:, c, ts(kt, 512)],
                                 mur2B[:, ts(kt, 512)])
            nc.scalar.activation(out=g_t[:, c, ts(kt, 512)],
                                 in_=cen[:, c, ts(kt, 512)], func=AF.Gelu,
                                 bias=bln2_s[:, c:c + 1],
                                 scale=g2_s[:, c:c + 1])
    ppF_b.release()
    ppF_st.release()
    pr2.release()
    rowsF.release()
    pcen.release()
    phsq.release()
    phb.release()

    # ============ Phase G: FFN ============
    pf1 = tc.alloc_tile_pool(name="pf1", bufs=1)
    ppG = tc.alloc_tile_pool(name="ppG", bufs=8, space="PSUM")
    pw2 = tc.alloc_tile_pool(name="pw2", bufs=2)
    pw1 = tc.alloc_tile_pool(name="pw1", bufs=3)

    w2_first = pw2.tile([P, FC, P], BF16, tag="w2", name="w2t0")
    nc.sync.dma_start(out=w2_first, in_=io["w2"][0])
    f1g = pf1.tile([P, FC, Tq], BF16, name="f1g")
    for j in range(FC):
        w1_t = pw1.tile([P, DC, P], BF16, tag="w1", name=f"w1t{j}")
        nc.sync.dma_start(out=w1_t, in_=io["w1"][j])
        ps = [ppG.tile([P, 512], F32, tag="mm", name=f"psf{j}_{n}")
              for n in range(NQ)]
        for k in range(DC):
            for n in range(NQ):
                nc.tensor.matmul(ps[n], w1_t[:, k, :], g_t[:, k, ts(n, 512)],
                                 start=(k == 0), stop=(k == DC - 1))
        for n in range(NQ):
            nc.scalar.activation(out=f1g[:, j, ts(n, 512)], in_=ps[n],
                                 func=AF.Gelu, bias=b1_s[:, j:j + 1],
                                 scale=1.0)
    pw1.release()

    poutc = tc.alloc_tile_pool(name="poutc", bufs=2)
    for j in range(DC):
        if j == 0:
            w2_t = w2_first
        else:
            w2_t = pw2.tile([P, FC, P], BF16, tag="w2", name=f"w2t{j}")
            nc.sync.dma_start(out=w2_t, in_=io["w2"][j])
        ps = [ppG.tile([P, 512], F32, tag="mm", name=f"pso{j}_{n}")
              for n in range(NQ)]
        for k in range(FC):
            for n in range(NQ):
                nc.tensor.matmul(ps[n], w2_t[:, k, :], f1g[:, k, ts(n, 512)],
                                 start=(k == 0), stop=(k == FC - 1))
        outc = poutc.tile([P, Tq], F32, tag="oc", name=f"outc{j}")
        for n in range(NQ):
            nc.vector.scalar_tensor_tensor(
                out=outc[:, ts(n, 512)], in0=ps[n], scalar=b2_s[:, j:j + 1],
                in1=h_t[:, j, ts(n, 512)], op0=ALU.add, op1=ALU.add)
            nc.sync.dma_start(out=io["out"][j][:, ts(n, 512)],
                              in_=outc[:, ts(n, 512)])

    poutc.release()
    pw2.release()
    ppG.release()
    pf1.release()
    pg.release()
    ph.release()
    poch.release()
    consts.release()


# ----------------------------------------------------------------------------
# host side
# ----------------------------------------------------------------------------

def _stripe(v):
    """[n*P] -> [P, n] per-partition striping (feature f = c*P + p)."""
    v = np.asarray(v, np.float32)
    return np.ascontiguousarray(v.reshape(-1, P).T)


def _lhsT_stream(W):
    """[Din, Dout] -> [Dout/P, P, Din/P, P] so slice [j] is the lhsT stream
    tile [P(din), Din/P, P(dout cols)] with contiguous per-partition rows."""
    din, dout = W.shape
    r = W.astype(bf16).reshape(din // P, P, dout // P, P)
    return np.ascontiguousarray(r.transpose(2, 1, 0, 3))


def prep_shared(inputs):
    f32 = np.float32
    g1 = np.asarray(inputs["ln1_g"], f32)
    b1n = np.asarray(inputs["ln1_b"], f32)
    W_ap = np.asarray(inputs["W_ap"], f32)
    b_ap = np.asarray(inputs["b_ap"], f32)
    W_qkv = np.asarray(inputs["W_qkv"], f32)
    b_qkv = np.asarray(inputs["b_qkv"], f32)
    W_proj = np.asarray(inputs["W_proj"], f32)

    # fold LN1 gamma and the whole attn pre-projection into W_qkv:
    # qkv = ln1(x) @ W_ap' @ W_qkv + (b_ap' @ W_qkv + b_qkv)
    W_eff = (g1[:, None] * W_ap) @ W_qkv
    b_eff = (b_ap + b1n @ W_ap) @ W_qkv + b_qkv
    shared = {
        "wq": _lhsT_stream(W_eff[:, 0:D]),
        "wk": _lhsT_stream(W_eff[:, D:2 * D]),
        "wv": np.ascontiguousarray(
            W_eff[:, 2 * D:].astype(bf16).reshape(DC, P, D)),
        "bqkv": _stripe(b_eff[:2 * D]),
        "bv": np.ascontiguousarray(np.asarray(b_eff[2 * D:], f32)),
        "wproj": _lhsT_stream(W_proj),
        "bproj": _stripe(np.asarray(inputs["b_proj"], f32)),
        "w1": _lhsT_stream(np.asarray(inputs["W1"], f32)),
        "b1": _stripe(np.asarray(inputs["b1"], f32)),
        "w2": _lhsT_stream(np.asarray(inputs["W2"], f32)),
        "b2": _stripe(np.asarray(inputs["b2"], f32)),
        "g2": _stripe(np.asarray(inputs["ln2_g"], f32)),
        "bln2": _stripe(np.asarray(inputs["ln2_b"], f32)),
    }
    return shared


def prep_core_x(x, core):
    b, qh = core // 2, core % 2
    xTb = np.asarray(x[b], np.float32).T  # [D, T] view
    if qh:
        xTb = np.concatenate([xTb[:, Tq:], xTb[:, :Tq]], axis=1)
    return np.ascontiguousarray(xTb.astype(bf16).reshape(DC, P, T))


def assemble_output(results, dtype):
    out = np.empty((B, T, D), dtype)
    for c in range(N_CORES):
        b, qh = c // 2, c % 2
        arr = np.asarray(results[c]["out"]).reshape(D, Tq)
        out[b, qh * Tq:(qh + 1) * Tq, :] = arr.T
    return out


def kernel(**inputs):
    x = np.asarray(inputs["x"], np.float32)
    shared = prep_shared(inputs)
    nc = build_nc()
    in_maps = [dict(shared, xt=prep_core_x(x, c)) for c in range(N_CORES)]
    res = run_bass_kernel_spmd(nc, in_maps, list(range(N_CORES)))
    return assemble_output(res.results, np.float32)


if __name__ == "__main__":
    nc = build_nc()
    print("built ok")
